# revision 9
# baseline (speedup 1.0000x reference)
"""Trainium2 8-core kernel for nn_Attention_5342939316549.

Design (head-sharded tensor parallel, per sharding hint):
  - core r owns query head r (N=8 heads, K=1 shared KV head).
  - KV projection is row-sharded (each core computes K,V for 1/8 of the
    (b,t) rows), then AllGather'ed (K^T fp32, V bf16).
  - Q projection + RoPE + attention for the core's own head run fully
    on-core with float32r (single-pass reduced fp32) matmuls.
  - P (softmax probs) is transposed through the PE array against a
    diag(1/rowsum) matrix, fusing softmax normalization into the
    transpose; P/V/AV run in bf16 (fp32 PSUM accumulation).
  - encoded^T is exchanged with an AllToAll so that every core ends up
    with all 8 heads for 4 of the 32 output row-tiles; the output
    projection (bf16 weights) then produces complete output rows on
    each core (no cross-core reduction needed).
  - host only shards/transposes inputs, builds RoPE tables, and
    concatenates the per-core output row-tiles.

Shapes (hardcoded from the problem spec):
  x0 [4,768,2048], x1 [4,256,1024], positions [4,1024],
  attn_mask [4,1,1024,1024], out = (out0 [4,768,2048], out1 [4,256,1024])
"""

import os
import sys

sys.path.insert(0, "/opt/trn_rl_repo")

import numpy as np
import ml_dtypes


def _install_ntff_hook_shim():
    """The agent image's antenv lacks axon_hooks; rebuild it so
    run_bass_kernel_spmd(trace=True) can capture NTFF profiles.
    Recipe mirrors trn_agent_boot/trn_boot.py::_ntff_profile_via_ctypes."""
    import types, ctypes, contextlib

    try:
        import antenv.axon_hooks  # noqa: F401
        return
    except ImportError:
        pass
    so_path = os.environ.get("PJRT_LIBRARY_PATH", "/opt/axon/libaxon_pjrt.so")
    hook = None
    try:
        lib = ctypes.CDLL(so_path)
        if hasattr(lib, "axon_start_nrt_profile"):
            lib.axon_start_nrt_profile.argtypes = [
                ctypes.POINTER(ctypes.c_int64), ctypes.c_size_t]
            lib.axon_start_nrt_profile.restype = ctypes.c_int64
            lib.axon_stop_nrt_profile.argtypes = [ctypes.c_char_p]
            lib.axon_stop_nrt_profile.restype = ctypes.c_int64

            @contextlib.contextmanager
            def _hook(output_dir, device_ids):
                import jax
                jax.devices()
                if device_ids:
                    ids = (ctypes.c_int64 * len(device_ids))(*device_ids)
                    rc = lib.axon_start_nrt_profile(ids, len(device_ids))
                else:
                    rc = lib.axon_start_nrt_profile(None, 0)
                if rc != 0:
                    raise RuntimeError(f"axon_start_nrt_profile rc={rc}")
                try:
                    yield
                finally:
                    n = lib.axon_stop_nrt_profile(str(output_dir).encode())
                    print(f"ntff profile: {n} file(s) -> {output_dir}",
                          file=sys.stderr)
            hook = _hook
    except OSError:
        pass
    mod = types.ModuleType("antenv.axon_hooks")
    mod._hook = hook
    mod.get_axon_ntff_profile_hook = lambda: mod._hook
    mod.set_axon_ntff_profile_hook = lambda h: setattr(mod, "_hook", h)
    sys.modules["antenv.axon_hooks"] = mod


_install_ntff_hook_shim()

import concourse.bass as bass
import concourse.mybir as mybir
from concourse import bacc
from concourse import tile
from concourse.bass_utils import run_bass_kernel_spmd
from concourse.masks import make_identity

F32 = mybir.dt.float32
F32R = mybir.dt.float32r
BF16 = mybir.dt.bfloat16

B, S0, S1 = 4, 768, 256
T = S0 + S1                      # 1024
D0, D1 = 2048, 1024
N, H = 8, 256                    # query heads, head dim
HALF = H // 2                    # 128
P = 128                          # partition size
NT = T // P                      # 8 t-tiles per batch
R0, R1 = B * S0, B * S1          # 3072, 1024
RT = B * T                       # 4096
NC = 8                           # cores
KR0, KR1 = R0 // NC, R1 // NC    # 384, 128 kv-shard rows per core
KR = KR0 + KR1                   # 512
KR1P = 256                       # padded x1 kv slice width (f32r N>=256)
BIG_NEG = -2.3819763e38
QSCALE = float(H) ** -0.5
MAXWAV = 10000.0

_CACHE = {}
LAST_EXEC_NS = None


# ----------------------------------------------------------------- host utils

def _rope_tables(pos_flat):
    """pos_flat [L] -> cos,sin [128, L] f32 (h' on partitions)."""
    freq_exp = (2.0 / H) * np.arange(HALF, dtype=np.float64)
    inv_ts = MAXWAV ** -freq_exp                       # [128]
    rad = inv_ts[:, None] * pos_flat[None, :].astype(np.float64)
    return (np.cos(rad).astype(np.float32),
            np.sin(rad).astype(np.float32))


def _classify_mask(attn_mask):
    """Block classification of the [B,1,T,T] bool mask into 128x128 tiles.

    Returns (s_ext, bias_idx, mbias):
      s_ext[b][i]   : number of s-tiles to compute for t-tile i of batch b
      bias_idx[b][i][j] : None (block all-true) or index into mbias
      mbias         : [nmix,128,128] f32 additive bias tiles (deduped)
    """
    m = np.asarray(attn_mask)[:, 0]                   # [B,T,T] bool
    s_ext = [[0] * NT for _ in range(B)]
    bias_idx = [[dict() for _ in range(NT)] for _ in range(B)]
    tiles = []
    seen = {}
    for b in range(B):
        for i in range(NT):
            rows = m[b, i * P:(i + 1) * P]
            ext = 0
            for j in range(NT):
                if rows[:, j * P:(j + 1) * P].any():
                    ext = j + 1
            assert ext > 0, "fully-masked query tile unsupported"
            s_ext[b][i] = ext
            for j in range(ext):
                blk = rows[:, j * P:(j + 1) * P]
                if blk.all():
                    bias_idx[b][i][j] = None
                    continue
                key = blk.tobytes()
                if key not in seen:
                    seen[key] = len(tiles)
                    tiles.append(
                        np.where(blk, 0.0, BIG_NEG).astype(np.float32))
                bias_idx[b][i][j] = seen[key]
    if not tiles:
        tiles.append(np.zeros((P, P), np.float32))
    mbias = np.stack(tiles)
    return s_ext, bias_idx, mbias


def _x0_chunk_splits(c0, width):
    """Split x0t col range [c0,c0+width) at batch (768) boundaries.

    Yields (off_in_chunk, qt_col, length)."""
    out = []
    c = c0
    while c < c0 + width:
        b = c // S0
        end = min((b + 1) * S0, c0 + width)
        out.append((c - c0, b * T + (c - b * S0), end - c))
        c = end
    return out


# out-tile ownership: core j owns out0 tiles {j, j+8, j+16} and out1 tile j.
def _slot_qt_col(j, k):
    """QT/ENC global column of out-tile slot k of core j."""
    if k < 3:
        g = j + 8 * k
        b, jj = g // 6, g % 6
        return b * T + jj * P
    b, jj = j // 2, j % 2
    return b * T + S0 + jj * P


# ------------------------------------------------------------------ bass build

def _build_nc(s_ext, bias_idx, nmix):
    nc = bacc.Bacc("TRN2")

    # ---- dram parameters (inputs)
    x0t = nc.declare_dram_parameter("x0t", [D0, R0], F32, isOutput=False)
    x1t = nc.declare_dram_parameter("x1t", [D1, R1], F32, isOutput=False)
    x0tk = nc.declare_dram_parameter("x0tk", [D0, KR0], F32, isOutput=False)
    x1tk = nc.declare_dram_parameter("x1tk", [D1, KR1P], F32, isOutput=False)
    q0w = nc.declare_dram_parameter("q0w", [D0, H], F32, isOutput=False)
    q1w = nc.declare_dram_parameter("q1w", [D1, H], F32, isOutput=False)
    kv0w = nc.declare_dram_parameter("kv0w", [2, D0, H], F32, isOutput=False)
    kv1w = nc.declare_dram_parameter("kv1w", [2, D1, H], F32, isOutput=False)
    o0w = nc.declare_dram_parameter("o0w", [N * H, D0], BF16, isOutput=False)
    o1w = nc.declare_dram_parameter("o1w", [N * H, D1], BF16, isOutput=False)
    cosg = nc.declare_dram_parameter("cosg", [P, RT], F32, isOutput=False)
    sing = nc.declare_dram_parameter("sing", [P, RT], F32, isOutput=False)
    cosk = nc.declare_dram_parameter("cosk", [P, KR], F32, isOutput=False)
    sink = nc.declare_dram_parameter("sink", [P, KR], F32, isOutput=False)
    mbias = nc.declare_dram_parameter("mbias", [nmix, P, P], F32,
                                      isOutput=False)
    out0p = nc.declare_dram_parameter("out0p", [3 * P, D0], F32, isOutput=True)
    out1p = nc.declare_dram_parameter("out1p", [P, D1], F32, isOutput=True)

    rg = [list(range(NC))]

    def r32(ap):
        return ap.bitcast(F32R)

    with tile.TileContext(nc) as tc:
        with (
            tc.tile_pool(name="dram", bufs=1, space="DRAM") as dramp,
            tc.tile_pool(name="const", bufs=1) as constp,
            tc.tile_pool(name="big", bufs=1) as bigp,
        ):
            # ---- dram bounce buffers
            k_send = dramp.tile([2, P, KR], F32R)
            k_ag = dramp.tile([NC, 2, P, KR], F32R, addr_space="Shared")
            v_send = dramp.tile([KR, H], BF16)
            v_ag = dramp.tile([NC, KR, H], BF16, addr_space="Shared")
            e_send = dramp.tile([NC, 2, P, 512], BF16)
            e_all = dramp.tile([NC, 2, P, 512], BF16)

            # ---- persistent sbuf
            QT = bigp.tile([P, 2, RT], F32R)     # Q^T (roped, scaled)
            KT = bigp.tile([P, 2, RT], F32R)     # K^T (roped, gathered)
            Vt = bigp.tile([P, 4 * NT, H], BF16)  # V rows (gathered)
            ENC = bigp.tile([P, 2, RT], BF16)   # encoded^T for own head

            ident = constp.tile([P, P], BF16)
            make_identity(nc, ident[:])
            mb = constp.tile([P, nmix, P], F32)
            nc.sync.dma_start(out=mb[:], in_=mbias[:].rearrange(
                "n p q -> p n q"))

            # ============== stage A: kv proj (shard) + AllGather ==========
            with (
                tc.tile_pool(name="kvw", bufs=4) as kvwp,
                tc.tile_pool(name="kvx", bufs=4) as kvxp,
                tc.tile_pool(name="kvs", bufs=1) as kvsp,
                tc.tile_pool(name="kvps", bufs=2, space="PSUM") as kvpsp,
            ):
                ksh = kvsp.tile([P, 2, KR], F32R)
                vtsh = kvsp.tile([P, 2, KR], BF16)
                ckt = kvsp.tile([P, KR], F32)
                skt = kvsp.tile([P, KR], F32)
                nc.sync.dma_start(out=ckt[:], in_=cosk[:])
                nc.sync.dma_start(out=skt[:], in_=sink[:])

                kraw = kvsp.tile([P, 2, KR], F32)
                for c in range(2):          # 0=k, 1=v
                    for ht in range(2):
                        # bank 0: x0 rows at [0:384); bank 1: x1 rows at
                        # [512:768) (matmul may not cross a psum bank)
                        ps = kvpsp.tile([P, 1024], F32)
                        for kt in range(D0 // P):
                            w = kvwp.tile([P, P], F32R)
                            nc.sync.dma_start(
                                out=w[:],
                                in_=kv0w[c, kt * P:(kt + 1) * P,
                                         ht * P:(ht + 1) * P].bitcast(F32R))
                            x = kvxp.tile([P, KR0], F32R, tag="kvx0")
                            nc.sync.dma_start(
                                out=x[:], in_=x0tk[kt * P:(kt + 1) * P, :].bitcast(F32R))
                            nc.tensor.matmul(
                                ps[:, 0:KR0], r32(w[:]), r32(x[:]),
                                start=(kt == 0), stop=(kt == D0 // P - 1))
                        for kt in range(D1 // P):
                            w = kvwp.tile([P, P], F32R)
                            nc.sync.dma_start(
                                out=w[:],
                                in_=kv1w[c, kt * P:(kt + 1) * P,
                                         ht * P:(ht + 1) * P].bitcast(F32R))
                            x = kvxp.tile([P, KR1P], F32R, tag="kvx1")
                            nc.sync.dma_start(
                                out=x[:], in_=x1tk[kt * P:(kt + 1) * P, :].bitcast(F32R))
                            nc.tensor.matmul(
                                ps[:, 512:512 + KR1P], r32(w[:]), r32(x[:]),
                                start=(kt == 0), stop=(kt == D1 // P - 1))
                        dst = kraw if c == 0 else vtsh
                        nc.scalar.copy(out=dst[:, ht, 0:KR0],
                                       in_=ps[:, 0:KR0])
                        nc.scalar.copy(out=dst[:, ht, KR0:KR],
                                       in_=ps[:, 512:512 + KR1])

                # RoPE on K shard: k0' = k0*c - k1*s ; k1' = k1*c + k0*s
                with tc.tile_pool(name="krt", bufs=1) as krtp:
                    t0 = krtp.tile([P, KR], F32)
                    t1 = krtp.tile([P, KR], F32)
                    t2 = krtp.tile([P, KR], F32)
                    t3 = krtp.tile([P, KR], F32)
                    nc.vector.tensor_mul(t0[:], kraw[:, 0], ckt[:])
                    nc.vector.tensor_mul(t1[:], kraw[:, 1], skt[:])
                    nc.vector.tensor_mul(t2[:], kraw[:, 1], ckt[:])
                    nc.vector.tensor_mul(t3[:], kraw[:, 0], skt[:])
                    nc.vector.tensor_sub(ksh[:, 0], t0[:], t1[:])
                    nc.vector.tensor_add(ksh[:, 1], t2[:], t3[:])

                nc.sync.dma_start(out=k_send[0], in_=ksh[:, 0])
                nc.sync.dma_start(out=k_send[1], in_=ksh[:, 1])

                # V shard -> natural [s,h] via PE transpose (bf16)
                for st in range(KR // P):
                    psv = kvpsp.tile([P, H], F32, tag="vps")
                    vtmp = kvxp.tile([P, H], BF16, tag="vtmp")
                    for ht in range(2):
                        nc.tensor.matmul(
                            psv[:, ht * P:(ht + 1) * P],
                            vtsh[:, ht, st * P:(st + 1) * P], ident[:])
                    nc.scalar.copy(out=vtmp[:], in_=psv[:])
                    nc.sync.dma_start(
                        out=v_send[st * P:(st + 1) * P, :], in_=vtmp[:])

            nc.gpsimd.collective_compute(
                "AllGather", mybir.AluOpType.bypass, replica_groups=rg,
                ins=[k_send[:].opt()], outs=[k_ag[:].opt()])
            nc.gpsimd.collective_compute(
                "AllGather", mybir.AluOpType.bypass, replica_groups=rg,
                ins=[v_send[:].opt()], outs=[v_ag[:].opt()])

            # ============== stage B: q proj + RoPE (overlaps AG) ==========
            with (
                tc.tile_pool(name="qw", bufs=4) as qwp,
                tc.tile_pool(name="qx", bufs=8) as qxp,
                tc.tile_pool(name="qps", bufs=6, space="PSUM") as qpsp,
                tc.tile_pool(name="tab", bufs=1) as tabp,
            ):
                cg = tabp.tile([P, RT], F32)
                sg = tabp.tile([P, RT], F32)
                nc.sync.dma_start(out=cg[:], in_=cosg[:])
                nc.sync.dma_start(out=sg[:], in_=sing[:])

                # x0 cols: 6 chunks of 512, in 2 groups of 3
                for grp in range(2):
                    ccs = [grp * 3 + t for t in range(3)]
                    pss = {}
                    for cc in ccs:
                        for ht in range(2):
                            pss[(cc, ht)] = qpsp.tile(
                                [P, 512], F32, tag="qps",
                                name=f"qps_{cc}_{ht}")
                    for kt in range(D0 // P):
                        xts = {}
                        for cc in ccs:
                            x = qxp.tile([P, 512], F32R, tag="qx")
                            nc.sync.dma_start(
                                out=x[:],
                                in_=x0t[kt * P:(kt + 1) * P,
                                        cc * 512:(cc + 1) * 512].bitcast(F32R))
                            xts[cc] = x
                        for ht in range(2):
                            w = qwp.tile([P, P], F32R)
                            nc.sync.dma_start(
                                out=w[:],
                                in_=q0w[kt * P:(kt + 1) * P,
                                        ht * P:(ht + 1) * P].bitcast(F32R))
                            for cc in ccs:
                                nc.tensor.matmul(
                                    pss[(cc, ht)][:], r32(w[:]),
                                    r32(xts[cc][:]),
                                    start=(kt == 0),
                                    stop=(kt == D0 // P - 1))
                    for cc in ccs:
                        for ht in range(2):
                            for off, qc, ln in _x0_chunk_splits(cc * 512, 512):
                                nc.scalar.mul(
                                    out=QT[:, ht, qc:qc + ln],
                                    in_=pss[(cc, ht)][:, off:off + ln],
                                    mul=QSCALE)

                # x1 cols: 4 chunks of 256 (batch-aligned), 2 groups of 2
                for grp in range(2):
                    ccs = [grp * 2 + t for t in range(2)]
                    pss = {}
                    for cc in ccs:
                        for ht in range(2):
                            pss[(cc, ht)] = qpsp.tile(
                                [P, 256], F32, tag="qps",
                                name=f"qps1_{cc}_{ht}")
                    for kt in range(D1 // P):
                        xts = {}
                        for cc in ccs:
                            x = qxp.tile([P, 256], F32R, tag="qx")
                            nc.sync.dma_start(
                                out=x[:],
                                in_=x1t[kt * P:(kt + 1) * P,
                                        cc * 256:(cc + 1) * 256].bitcast(F32R))
                            xts[cc] = x
                        for ht in range(2):
                            w = qwp.tile([P, P], F32R)
                            nc.sync.dma_start(
                                out=w[:],
                                in_=q1w[kt * P:(kt + 1) * P,
                                        ht * P:(ht + 1) * P].bitcast(F32R))
                            for cc in ccs:
                                nc.tensor.matmul(
                                    pss[(cc, ht)][:], r32(w[:]),
                                    r32(xts[cc][:]),
                                    start=(kt == 0),
                                    stop=(kt == D1 // P - 1))
                    for cc in ccs:
                        b = cc
                        for ht in range(2):
                            nc.scalar.mul(
                                out=QT[:, ht, b * T + S0:b * T + S0 + 256],
                                in_=pss[(cc, ht)][:], mul=QSCALE)

                # RoPE on Q (per batch)
                with tc.tile_pool(name="qrt", bufs=2) as qrtp:
                    for b in range(B):
                        bc = slice(b * T, (b + 1) * T)
                        t0 = qrtp.tile([P, T], F32, tag="r0")
                        t1 = qrtp.tile([P, T], F32, tag="r1")
                        t2 = qrtp.tile([P, T], F32, tag="r2")
                        t3 = qrtp.tile([P, T], F32, tag="r3")
                        nc.vector.tensor_mul(t0[:], QT[:, 0, bc], cg[:, bc])
                        nc.vector.tensor_mul(t1[:], QT[:, 1, bc], sg[:, bc])
                        nc.vector.tensor_mul(t2[:], QT[:, 1, bc], cg[:, bc])
                        nc.vector.tensor_mul(t3[:], QT[:, 0, bc], sg[:, bc])
                        nc.vector.tensor_sub(QT[:, 0, bc], t0[:], t1[:])
                        nc.vector.tensor_add(QT[:, 1, bc], t2[:], t3[:])

            # ============== stage C: assemble K^T / V from AG =============
            for r in range(NC):
                b, half = r // 2, r % 2
                d0c = b * T + KR0 * half
                d1c = b * T + S0 + KR1 * half
                for ht in range(2):
                    nc.sync.dma_start(out=KT[:, ht, d0c:d0c + KR0],
                                      in_=k_ag[r, ht, :, 0:KR0])
                    nc.sync.dma_start(out=KT[:, ht, d1c:d1c + KR1],
                                      in_=k_ag[r, ht, :, KR0:KR])
                st0 = d0c // P
                st1 = d1c // P
                nc.sync.dma_start(
                    out=Vt[:, st0:st0 + 3, :],
                    in_=v_ag[r, 0:KR0, :].rearrange("(st p) h -> p st h",
                                                    p=P))
                nc.sync.dma_start(out=Vt[:, st1, :], in_=v_ag[r, KR0:KR, :])

            # ============== stage D: attention ============================
            with (
                tc.tile_pool(name="lps", bufs=2, space="PSUM") as lpsp,
                tc.tile_pool(name="tps", bufs=1, space="PSUM") as tpsp,
                tc.tile_pool(name="aps", bufs=2, space="PSUM") as apsp,
                tc.tile_pool(name="pp", bufs=3) as ppp,
                tc.tile_pool(name="ptp", bufs=2) as ptp,
                tc.tile_pool(name="st", bufs=8) as stp,
            ):
                for b in range(B):
                    for c in range(2):          # t-chunks of 512
                        tis = [4 * c + u for u in range(4)]
                        jm = max(s_ext[b][i] for i in tis)
                        PT = ptp.tile([P, NT, 512], BF16, tag="pt")
                        for i in tis:
                            for j in range(s_ext[b][i], jm):
                                nc.vector.memset(
                                    PT[:, j, (i - 4 * c) * P:
                                       (i - 4 * c + 1) * P], 0.0)
                        for i in tis:
                            ext = s_ext[b][i]
                            scol = ext * P
                            tc0 = b * T + i * P
                            pl = lpsp.tile([P, T], F32, tag="lg")
                            for s0 in range(0, scol, 512):
                                sw = min(512, scol - s0)
                                for ht in range(2):
                                    nc.tensor.matmul(
                                        pl[:, s0:s0 + sw],
                                        r32(QT[:, ht, tc0:tc0 + P]),
                                        r32(KT[:, ht,
                                               b * T + s0:b * T + s0 + sw]),
                                        start=(ht == 0), stop=(ht == 1))
                            for j in range(ext):
                                bi = bias_idx[b][i].get(j)
                                if bi is not None:
                                    nc.vector.tensor_add(
                                        pl[:, j * P:(j + 1) * P],
                                        pl[:, j * P:(j + 1) * P],
                                        mb[:, bi, :])
                            Pr = ppp.tile([P, T], BF16, tag="p")
                            rs = stp.tile([P, 1], F32, tag="rs")
                            rc = stp.tile([P, 1], F32, tag="rc")
                            nc.scalar.activation(
                                Pr[:, 0:scol], pl[:, 0:scol],
                                mybir.ActivationFunctionType.Exp,
                                accum_out=rs[:])
                            nc.vector.reciprocal(rc[:], rs[:])
                            dg = stp.tile([P, P], BF16, tag="dg")
                            nc.vector.tensor_scalar_mul(dg[:], ident[:],
                                                        rc[:])
                            # transpose P^T * diag(1/sum) via PE
                            pt_ps = tpsp.tile([P, T], F32, tag="tp")
                            for j in range(ext):
                                nc.tensor.matmul(
                                    pt_ps[:, j * P:(j + 1) * P],
                                    Pr[:, j * P:(j + 1) * P], dg[:])
                            nc.scalar.copy(
                                out=PT[:, 0:ext,
                                       (i - 4 * c) * P:(i - 4 * c + 1) * P],
                                in_=pt_ps[:, 0:scol].rearrange(
                                    "p (j q) -> p j q", q=P))
                        # AV: enc^T[h, t-chunk]
                        for ht in range(2):
                            pe = apsp.tile([P, 512], F32, tag="av")
                            for j in range(jm):
                                nc.tensor.matmul(
                                    pe[:],
                                    Vt[:, b * NT + j, ht * P:(ht + 1) * P],
                                    PT[:, j, :],
                                    start=(j == 0), stop=(j == jm - 1))
                            oc = b * T + c * 512
                            nc.scalar.copy(out=ENC[:, ht, oc:oc + 512],
                                           in_=pe[:])

            # ============== stage E: A2A of encoded^T =====================
            for j in range(NC):
                for k in range(4):
                    col = _slot_qt_col(j, k)
                    for ht in range(2):
                        nc.sync.dma_start(
                            out=e_send[j, ht, :, k * P:(k + 1) * P],
                            in_=ENC[:, ht, col:col + P])
            nc.gpsimd.collective_compute(
                "AllToAll", mybir.AluOpType.bypass, replica_groups=rg,
                ins=[e_send[:].opt()], outs=[e_all[:].opt()])

            # ============== stage F: output projection ====================
            with (
                tc.tile_pool(name="et", bufs=1) as etp,
                tc.tile_pool(name="ow", bufs=8) as owp,
                tc.tile_pool(name="ops", bufs=6, space="PSUM") as opsp,
                tc.tile_pool(name="ob", bufs=2) as obp,
            ):
                et = etp.tile([P, 4, N * 2, P], BF16)
                for k in range(4):
                    for n in range(N):
                        for ht in range(2):
                            nc.sync.dma_start(
                                out=et[:, k, 2 * n + ht, :],
                                in_=e_all[n, ht, :, k * P:(k + 1) * P])

                outs0 = [obp.tile([P, D0], F32, tag="ob", name=f"ob{k}")
                         for k in range(3)]
                for dc in range(4):           # one 512-wide d-chunk at a time
                    pss = {k: opsp.tile([P, 512], F32, tag="ops",
                                        name=f"ops_{k}_{dc}")
                           for k in range(3)}
                    for nh in range(16):
                        w = owp.tile([P, 512], BF16, tag="ow")
                        nc.sync.dma_start(
                            out=w[:],
                            in_=o0w[nh * P:(nh + 1) * P,
                                    dc * 512:(dc + 1) * 512])
                        for k in range(3):
                            nc.tensor.matmul(
                                pss[k][:], et[:, k, nh, :], w[:],
                                start=(nh == 0), stop=(nh == 15))
                    for k in range(3):
                        nc.scalar.copy(
                            out=outs0[k][:, dc * 512:(dc + 1) * 512],
                            in_=pss[k][:])
                for k in range(3):
                    nc.sync.dma_start(out=out0p[k * P:(k + 1) * P, :],
                                      in_=outs0[k][:])

                outs1 = obp.tile([P, D1], F32, tag="ob1")
                for dc in range(2):
                    ps1 = opsp.tile([P, 512], F32, tag="ops",
                                    name=f"ops1_{dc}")
                    for nh in range(16):
                        w = owp.tile([P, 512], BF16, tag="ow")
                        nc.sync.dma_start(
                            out=w[:], in_=o1w[nh * P:(nh + 1) * P,
                                              dc * 512:(dc + 1) * 512])
                        nc.tensor.matmul(
                            ps1[:], et[:, 3, nh, :], w[:],
                            start=(nh == 0), stop=(nh == 15))
                    nc.scalar.copy(out=outs1[:, dc * 512:(dc + 1) * 512],
                                   in_=ps1[:])
                nc.sync.dma_start(out=out1p[:], in_=outs1[:])

    nc.compile()
    return nc


# ------------------------------------------------------------------ entrypoint

def kernel(x0, x1, positions, attn_mask, q0_w, kv0_w, q1_w, kv1_w,
           o0_w, o1_w):
    global LAST_EXEC_NS
    x0 = np.asarray(x0, dtype=np.float32)
    x1 = np.asarray(x1, dtype=np.float32)
    positions = np.asarray(positions, dtype=np.float32)
    q0_w = np.asarray(q0_w, dtype=np.float32)
    kv0_w = np.asarray(kv0_w, dtype=np.float32)
    q1_w = np.asarray(q1_w, dtype=np.float32)
    kv1_w = np.asarray(kv1_w, dtype=np.float32)
    o0_w = np.asarray(o0_w, dtype=np.float32)
    o1_w = np.asarray(o1_w, dtype=np.float32)

    s_ext, bias_idx, mbias = _classify_mask(attn_mask)
    nmix = mbias.shape[0]

    key = (repr(s_ext),
           repr([[sorted(d.items()) for d in row] for row in bias_idx]),
           nmix)
    if key not in _CACHE:
        _CACHE[key] = _build_nc(s_ext, bias_idx, nmix)
    nc = _CACHE[key]

    # host-side shard prep
    x0t = np.ascontiguousarray(x0.reshape(R0, D0).T)        # [D0, R0]
    x1t = np.ascontiguousarray(x1.reshape(R1, D1).T)        # [D1, R1]
    # global (b,t) flattened positions
    pos_flat = positions.reshape(RT)
    cosg, sing = _rope_tables(pos_flat)
    o0wb = o0_w.reshape(N * H, D0).astype(ml_dtypes.bfloat16)
    o1wb = o1_w.reshape(N * H, D1).astype(ml_dtypes.bfloat16)
    kv0 = np.ascontiguousarray(kv0_w[:, 0])                 # [2, D0, H]
    kv1 = np.ascontiguousarray(kv1_w[:, 0])

    in_maps = []
    for r in range(NC):
        # kv shard rows for this core
        r0s = np.arange(KR0 * r, KR0 * (r + 1))
        r1s = np.arange(KR1 * r, KR1 * (r + 1))
        posk = np.concatenate([
            positions[r0s // S0, r0s % S0],
            positions[r1s // S1, S0 + (r1s % S1)]])
        ck, sk = _rope_tables(posk)
        x1tk = np.zeros((D1, KR1P), np.float32)
        x1tk[:, 0:KR1] = x1t[:, KR1 * r:KR1 * (r + 1)]
        in_maps.append({
            "x0t": x0t,
            "x1t": x1t,
            "x0tk": np.ascontiguousarray(x0t[:, KR0 * r:KR0 * (r + 1)]),
            "x1tk": x1tk,
            "q0w": np.ascontiguousarray(q0_w[r]),
            "q1w": np.ascontiguousarray(q1_w[r]),
            "kv0w": kv0,
            "kv1w": kv1,
            "o0w": o0wb,
            "o1w": o1wb,
            "cosg": cosg,
            "sing": sing,
            "cosk": ck,
            "sink": sk,
            "mbias": mbias,
        })

    res = run_bass_kernel_spmd(
        nc, in_maps, core_ids=list(range(NC)),
        trace=os.environ.get("KTRACE", "0") == "1")
    LAST_EXEC_NS = res.exec_time_ns

    out0 = np.empty((B, S0, D0), np.float32)
    out1 = np.empty((B, S1, D1), np.float32)
    for r in range(NC):
        o0p = res.results[r]["out0p"]
        o1p = res.results[r]["out1p"]
        for k in range(3):
            g = r + 8 * k
            b, jj = g // 6, g % 6
            out0[b, jj * P:(jj + 1) * P, :] = o0p[k * P:(k + 1) * P]
        out1[r // 2, (r % 2) * P:(r % 2 + 1) * P, :] = o1p
    return out0, out1
